# revision 5
# baseline (speedup 1.0000x reference)
"""2-layer GAT on 8 TRN2 NeuronCores.

Strategy (per-edge random access is unavailable in this environment — the
extended dma_gather ucode crashes and indirect DMA runs ~1.4us/128 rows — so
all device memory traffic is sequential streams; per-edge irregularity is
encoded host-side from edge_index into streams, and segment softmax/reduce
run on the PE via one-hot staircase matmuls):

  Launch A (1-D node shard): h1 = x @ W1, as1/ad1 attention halves -> tables.
  Host: permute tables into dst-sorted per-edge streams (layout only).
  Launch B (1-D dst shard): p = exp(lrelu(as+ad)) per edge; W_all = [p*h | p];
    per 128-edge chunk matmul with one-hot M^T gives segment sums S|z in PSUM;
    epilogue: out1 = S/z + b1, elu, h2/as2/ad2 tables for layer 2.
  Host: permute layer-2 tables into streams.
  Launch C: same machinery with H=1, C=7; log_softmax; output shard.

Numerics: segment-softmax max-subtraction is skipped (logit scale here is
~|e|<2 so exp is safe); softmax is alpha = p / sum(p), identical math.
"""
import numpy as np

import concourse.bass as bass
import concourse.mybir as mybir
import concourse.tile as tile
from concourse import bacc
from concourse.masks import make_identity
from concourse.bass_utils import run_bass_kernel_spmd

F32 = mybir.dt.float32
BF16 = mybir.dt.bfloat16
AF = mybir.ActivationFunctionType

N = 100000
E = 1600000
F_IN = 512
H = 8
D = 8
HD = 64
C = 7
NEG = 0.2
NCORES = 8
NSHARD = N // NCORES          # 12500
P = 128
NTILE = (NSHARD + P - 1) // P  # 98
NPAD = NTILE * P               # 12544
WIN = 64
NSLOT = NTILE * 2              # 196
KSUP = 64                      # chunks per superchunk
R1 = 72                        # [p*h(64) | p(8)]
R2 = 8                         # [p*h2(7) | p(1)]
STAGE_G = 14                   # tiles per output staging flush


# ---------------------------------------------------------------- host prep

def build_structure(edge_index):
    """Edge_index-derived structure. Returns shared chunk metadata and
    per-core edge placements."""
    src = np.concatenate([edge_index[0], np.arange(N, dtype=np.int64)]).astype(np.int64)
    dst = np.concatenate([edge_index[1], np.arange(N, dtype=np.int64)]).astype(np.int64)

    cores = []
    counts = np.zeros((NCORES, NSLOT), np.int64)
    for k in range(NCORES):
        lo = k * NSHARD
        sel = (dst >= lo) & (dst < lo + NSHARD)
        s_k = src[sel].astype(np.int32)
        d_k = (dst[sel] - lo).astype(np.int32)
        order = np.argsort(d_k, kind="stable")
        s_k, d_k = s_k[order], d_k[order]
        slot = d_k >> 6  # 64-node windows = slots
        counts[k] = np.bincount(slot, minlength=NSLOT)
        cores.append((s_k, d_k, slot))

    cs = np.maximum(1, -(-counts.max(axis=0) // P))  # chunks per slot (shared)
    kt_real = int(cs.sum())
    kt = -(-kt_real // KSUP) * KSUP               # pad to superchunk multiple
    n_trash = kt - kt_real

    # shared chunk metadata
    chunk_tile = np.empty(kt, np.int32)
    chunk_b = np.empty(kt, np.int32)
    chunk_start = np.zeros(kt, bool)
    chunk_stop = np.zeros(kt, bool)
    chunk_epi = np.full(kt, -1, np.int32)  # tile to epilogue after this chunk
    slot_off = np.zeros(NSLOT + 1, np.int64)
    c = 0
    for s in range(NSLOT):
        t, w = s >> 1, s & 1
        slot_off[s] = c
        for j in range(int(cs[s])):
            chunk_tile[c] = t
            chunk_b[c] = w * WIN
            chunk_start[c] = j == 0
            chunk_stop[c] = j == int(cs[s]) - 1
            c += 1
        if w == 1:
            chunk_epi[c - 1] = t
    slot_off[NSLOT] = c
    assert c == kt_real
    chunk_tile[kt_real:] = -1  # trash chunks

    # per-core edge placement: position of edge i in the padded stream
    placements = []
    for k in range(NCORES):
        s_k, d_k, slot = cores[k]
        pos = np.empty(len(s_k), np.int64)
        cnt = np.bincount(slot, minlength=NSLOT)
        starts = slot_off[:-1] * P
        run = np.zeros(NSLOT, np.int64)
        # edges are slot-sorted; within slot keep order
        idx_in_slot = np.arange(len(s_k)) - np.concatenate(
            [[0], np.cumsum(cnt)])[slot]
        pos = starts[slot] + idx_in_slot
        placements.append((s_k, d_k, pos.astype(np.int64)))

    wloc_streams = []
    for k in range(NCORES):
        s_k, d_k, pos = placements[k]
        wl = np.zeros(kt * P, np.float32)
        wl[pos] = (d_k - (d_k >> 6 << 6)).astype(np.float32)
        import ml_dtypes
        wloc_streams.append(to_stream(wl[:, None], kt, 1).astype(ml_dtypes.bfloat16))

    meta = dict(kt=kt, kt_real=kt_real, chunk_tile=chunk_tile, chunk_b=chunk_b,
                chunk_start=chunk_start, chunk_stop=chunk_stop,
                chunk_epi=chunk_epi)
    return meta, placements, wloc_streams


def to_stream(arr, kt, w):
    """[kt*128, w] -> [128, kt*w] (edge j = c*128 + p -> [p, c*w:(c+1)*w])."""
    return np.ascontiguousarray(
        arr.reshape(kt, P, w).transpose(1, 0, 2).reshape(P, kt * w))


def make_streams(placements, kt, tab_h, tab_s, tab_ad, wh, ws):
    """Gathered per-edge streams from node tables (host layout op)."""
    ghs, sss, ads = [], [], []
    for k in range(NCORES):
        s_k, d_k, pos = placements[k]
        lo = k * NSHARD
        import ml_dtypes
        gh = np.zeros((kt * P, wh), np.float32)
        gh[pos] = tab_h[s_k]
        gh = gh.astype(ml_dtypes.bfloat16)
        ss = np.full((kt * P, ws), -1e9, np.float32)
        ss[pos] = tab_s[s_k]
        ss = ss.astype(ml_dtypes.bfloat16)
        ad = np.zeros((kt * P, ws), np.float32)
        ad[pos] = tab_ad[lo + d_k]
        ad = ad.astype(ml_dtypes.bfloat16)
        ghs.append(to_stream(gh, kt, wh))
        sss.append(to_stream(ss, kt, ws))
        ads.append(to_stream(ad, kt, ws))
    return ghs, sss, ads


# ---------------------------------------------------------------- launch A

def build_A(reps=1):
    nc = bacc.Bacc("TRN2", target_bir_lowering=False)
    xt_in = nc.dram_tensor("XT", [P, 4 * NSHARD], F32, kind="ExternalInput")
    w1_in = nc.dram_tensor("W1", [F_IN, HD], F32, kind="ExternalInput")
    a1s_in = nc.dram_tensor("A1S", [HD], F32, kind="ExternalInput")
    a1d_in = nc.dram_tensor("A1D", [HD], F32, kind="ExternalInput")
    th_out = nc.dram_tensor("TH", [NPAD, HD], F32, kind="ExternalOutput")
    ts_out = nc.dram_tensor("TS", [NPAD, H], F32, kind="ExternalOutput")
    tad_out = nc.dram_tensor("TAD", [NPAD, H], F32, kind="ExternalOutput")

    GT = 7  # tile groups of STAGE_G
    with tile.TileContext(nc) as tc:
        with (
            tc.tile_pool(name="const", bufs=1) as cpool,
            tc.tile_pool(name="xt", bufs=2) as xpool,
            tc.tile_pool(name="st", bufs=2) as spool,
            tc.tile_pool(name="tmp", bufs=3) as tpool,
            tc.tile_pool(name="ps", bufs=2, space="PSUM") as ppool,
        ):
            w1 = cpool.tile([P, 4 * HD], F32)
            nc.sync.dma_start(
                out=w1[:].rearrange("k (c n) -> k c n", c=4),
                in_=w1_in[:, :].rearrange("(c k) n -> k c n", k=P))
            a1s = cpool.tile([P, HD], F32)
            nc.sync.dma_start(out=a1s[:], in_=a1s_in[None, :].to_broadcast([P, HD]))
            a1d = cpool.tile([P, HD], F32)
            nc.sync.dma_start(out=a1d[:], in_=a1d_in[None, :].to_broadcast([P, HD]))

            xt_d = xt_in[:, :].rearrange("k (c n) -> k c n", c=4)
            TILES_PER_DMA = 13
            nbuf = -(-NTILE // TILES_PER_DMA)

            sh = ss_ = sad = None
            for rep in range(reps):
              for t in range(NTILE):
                  if t % TILES_PER_DMA == 0:
                      ncols = min(TILES_PER_DMA * P, NSHARD - t * P)
                      xbuf = xpool.tile([P, 4 * TILES_PER_DMA * P], F32, tag="xbuf")
                      xv = xbuf[:].rearrange("k (c n) -> k c n", c=4)
                      nc.sync.dma_start(
                          out=xv[:, :, 0:ncols],
                          in_=xt_d[:, :, t * P:t * P + ncols])
                  if t % STAGE_G == 0:
                      sh = spool.tile([P, STAGE_G * HD], F32, tag="sh")
                      ss_ = spool.tile([P, STAGE_G * H], F32, tag="ss")
                      sad = spool.tile([P, STAGE_G * H], F32, tag="sad")
                  g = t % STAGE_G
                  rows = min(P, NSHARD - t * P)
                  lc = (t % TILES_PER_DMA) * P
                  ps = ppool.tile([P, HD], F32)
                  for cchunk in range(4):
                      nc.tensor.matmul(
                          ps[0:rows, :],
                          xv[:, cchunk, lc:lc + rows],
                          w1[:, cchunk * HD:(cchunk + 1) * HD],
                          start=(cchunk == 0), stop=(cchunk == 3))
                  hcol = sh[:, g * HD:(g + 1) * HD]
                  nc.vector.tensor_copy(out=hcol, in_=ps[:])
                  tmp = tpool.tile([P, HD], F32, tag="tmp")
                  nc.vector.tensor_tensor(out=tmp[:], in0=ps[:], in1=a1s[:],
                                          op=mybir.AluOpType.mult)
                  nc.vector.reduce_sum(
                      out=ss_[:, g * H:(g + 1) * H],
                      in_=tmp[:].rearrange("p (h d) -> p h d", h=H),
                      axis=mybir.AxisListType.X)
                  nc.vector.tensor_tensor(out=tmp[:], in0=ps[:], in1=a1d[:],
                                          op=mybir.AluOpType.mult)
                  nc.vector.reduce_sum(
                      out=sad[:, g * H:(g + 1) * H],
                      in_=tmp[:].rearrange("p (h d) -> p h d", h=H),
                      axis=mybir.AxisListType.X)
                  if g == STAGE_G - 1 or t == NTILE - 1:
                      g0 = t - g
                      ng = g + 1
                      nc.sync.dma_start(
                          out=th_out[g0 * P:(g0 + ng) * P, :].rearrange(
                              "(g p) c -> p g c", p=P),
                          in_=sh[:, 0:ng * HD].rearrange("p (g c) -> p g c", g=ng))
                      nc.sync.dma_start(
                          out=ts_out[g0 * P:(g0 + ng) * P, :].rearrange(
                              "(g p) c -> p g c", p=P),
                          in_=ss_[:, 0:ng * H].rearrange("p (g c) -> p g c", g=ng))
                      nc.sync.dma_start(
                          out=tad_out[g0 * P:(g0 + ng) * P, :].rearrange(
                              "(g p) c -> p g c", p=P),
                          in_=sad[:, 0:ng * H].rearrange("p (g c) -> p g c", g=ng))
    nc.compile()
    return nc


# ---------------------------------------------------------------- launch B/C

def build_edge_launch(meta, layer, reps=1, nomm=False):
    """layer 1: R=72 (8 heads), outputs T2 tables.
    layer 2: R=8 (1 head), outputs log-softmax shard."""
    kt = meta["kt"]
    nsup = kt // KSUP
    wh = HD if layer == 1 else C            # gathered h width
    ws = H if layer == 1 else 1             # as/ad width
    R = R1 if layer == 1 else R2

    nc = bacc.Bacc("TRN2", target_bir_lowering=False)
    gh_in = nc.dram_tensor("GH", [P, kt * wh], BF16, kind="ExternalInput")
    ss_in = nc.dram_tensor("SS", [P, kt * ws], BF16, kind="ExternalInput")
    ad_in = nc.dram_tensor("AD", [P, kt * ws], BF16, kind="ExternalInput")
    wl_in = nc.dram_tensor("WL", [P, kt], BF16, kind="ExternalInput")
    if layer == 1:
        b1_in = nc.dram_tensor("B1", [HD], F32, kind="ExternalInput")
        w2_in = nc.dram_tensor("W2", [HD, C], F32, kind="ExternalInput")
        a2s_in = nc.dram_tensor("A2S", [C], F32, kind="ExternalInput")
        a2d_in = nc.dram_tensor("A2D", [C], F32, kind="ExternalInput")
        t2_out = nc.dram_tensor("T2", [NPAD, 9], F32, kind="ExternalOutput")
    else:
        b2_in = nc.dram_tensor("B2", [C], F32, kind="ExternalInput")
        out_out = nc.dram_tensor("OUT", [NPAD, C], F32, kind="ExternalOutput")

    with tile.TileContext(nc) as tc:
        with (
            tc.tile_pool(name="const", bufs=1) as cpool,
            tc.tile_pool(name="stream", bufs=3) as dpool,
            tc.tile_pool(name="work", bufs=3) as wpool,
            tc.tile_pool(name="epi", bufs=2) as epool,
            tc.tile_pool(name="stage", bufs=2) as spool,
            tc.tile_pool(name="ps", bufs=2, space="PSUM") as ppool,
            tc.tile_pool(name="trashp", bufs=1, space="PSUM") as trpool,
            tc.tile_pool(name="pst", bufs=2, space="PSUM") as ptpool,
        ):
            iota_i = cpool.tile([P, WIN], mybir.dt.int32)
            nc.gpsimd.iota(iota_i[:], pattern=[[1, WIN]], base=0,
                           channel_multiplier=0)
            iota_f = cpool.tile([P, WIN], BF16)
            nc.vector.tensor_copy(out=iota_f[:], in_=iota_i[:])
            ident = cpool.tile([P, P], F32)
            make_identity(nc, ident[:])
            if layer == 1:
                b1r = cpool.tile([P, HD], F32)
                nc.sync.dma_start(out=b1r[:],
                                  in_=b1_in[None, :].to_broadcast([P, HD]))
                w2 = cpool.tile([HD, C], F32)
                nc.sync.dma_start(out=w2[:], in_=w2_in[:, :])
                a2sr = cpool.tile([HD, C], F32)
                nc.sync.dma_start(out=a2sr[:],
                                  in_=a2s_in[None, :].to_broadcast([HD, C]))
                a2dr = cpool.tile([HD, C], F32)
                nc.sync.dma_start(out=a2dr[:],
                                  in_=a2d_in[None, :].to_broadcast([HD, C]))
                # W2cat = [W2 | W2 @ a2s^T | W2 @ a2d^T]  ([64, 9])
                w2cat = cpool.tile([HD, 9], F32)
                nc.vector.tensor_copy(out=w2cat[:, 0:C], in_=w2[:])
                tmpw = cpool.tile([HD, C], F32)
                nc.vector.tensor_tensor(out=tmpw[:], in0=w2[:], in1=a2sr[:],
                                        op=mybir.AluOpType.mult)
                nc.vector.reduce_sum(out=w2cat[:, C:C + 1], in_=tmpw[:],
                                     axis=mybir.AxisListType.X)
                nc.vector.tensor_tensor(out=tmpw[:], in0=w2[:], in1=a2dr[:],
                                        op=mybir.AluOpType.mult)
                nc.vector.reduce_sum(out=w2cat[:, C + 1:C + 2], in_=tmpw[:],
                                     axis=mybir.AxisListType.X)
            else:
                b2r = cpool.tile([P, C], F32)
                nc.sync.dma_start(out=b2r[:],
                                  in_=b2_in[None, :].to_broadcast([P, C]))

            trash = trpool.tile([P, R], F32, tag="trash")
            trash_used = [False]
            stage = {"tile": None, "g0": 0}

            chunk_tile = meta["chunk_tile"]
            chunk_b = meta["chunk_b"]
            chunk_start = meta["chunk_start"]
            chunk_stop = meta["chunk_stop"]
            chunk_epi = meta["chunk_epi"]

            psum_by_tile = {}
            stage_tile = [None]
            stage_cols = 9 if layer == 1 else C

            def flush_stage(t_last):
                g0 = stage["g0"]
                ng = t_last - g0 + 1
                st = stage_tile[0]
                out_t = t2_out if layer == 1 else out_out
                nc.sync.dma_start(
                    out=out_t[g0 * P:(g0 + ng) * P, :].rearrange(
                        "(g p) c -> p g c", p=P),
                    in_=st[:, 0:ng * stage_cols].rearrange(
                        "p (g c) -> p g c", g=ng))
                stage_tile[0] = None

            def epilogue(t, ps):
                # S = ps[:, 0:wh*...]; layout [p*h | p]
                nh = H if layer == 1 else 1
                dd = D if layer == 1 else C
                zrec = epool.tile([P, nh], F32, tag="zrec")
                nc.vector.reciprocal(out=zrec[:], in_=ps[:, wh:wh + nh])
                o1 = epool.tile([P, wh], F32, tag="o1")
                nc.vector.tensor_tensor(
                    out=o1[:].rearrange("p (h d) -> p h d", h=nh),
                    in0=ps[:, 0:wh].rearrange("p (h d) -> p h d", h=nh),
                    in1=zrec[:, :, None].to_broadcast([P, nh, dd]),
                    op=mybir.AluOpType.mult)
                if stage_tile[0] is None:
                    stage_tile[0] = spool.tile(
                        [P, STAGE_G * stage_cols], F32, tag="stage",
                        name=f"stage{t}")
                    stage["g0"] = t
                st = stage_tile[0]
                g = t - stage["g0"]
                if layer == 1:
                    # h = elu(o1 + b1); T2 = [h@W2 | h@W2a2s | h@W2a2d]
                    nc.vector.tensor_tensor(out=o1[:], in0=o1[:], in1=b1r[:],
                                            op=mybir.AluOpType.add)
                    mn = epool.tile([P, wh], F32, tag="mn")
                    nc.vector.tensor_scalar_min(out=mn[:], in0=o1[:], scalar1=0.0)
                    nc.scalar.activation(mn[:], mn[:], AF.Exp)
                    mx = epool.tile([P, wh], F32, tag="mx")
                    nc.vector.tensor_scalar_max(out=mx[:], in0=o1[:], scalar1=0.0)
                    nc.vector.tensor_tensor(out=o1[:], in0=mx[:], in1=mn[:],
                                            op=mybir.AluOpType.add)
                    nc.vector.tensor_scalar_add(out=o1[:], in0=o1[:], scalar1=-1.0)
                    trp = ptpool.tile([HD, P], F32, tag="trp")
                    nc.tensor.transpose(trp[:], o1[:], ident[:])
                    trs = epool.tile([HD, P], F32, tag="trs")
                    nc.vector.tensor_copy(out=trs[:], in_=trp[:])
                    h2p = ptpool.tile([P, 9], F32, tag="h2p")
                    nc.tensor.matmul(h2p[:], trs[:], w2cat[:],
                                     start=True, stop=True)
                    nc.vector.tensor_copy(
                        out=st[:, g * 9:(g + 1) * 9], in_=h2p[:])
                else:
                    # log_softmax(o1 + b2)
                    nc.vector.tensor_tensor(out=o1[:], in0=o1[:], in1=b2r[:],
                                            op=mybir.AluOpType.add)
                    mmax = epool.tile([P, 1], F32, tag="mmax")
                    nc.vector.reduce_max(out=mmax[:], in_=o1[:],
                                         axis=mybir.AxisListType.X)
                    nc.vector.tensor_tensor(
                        out=o1[:], in0=o1[:],
                        in1=mmax[:].to_broadcast([P, C]),
                        op=mybir.AluOpType.subtract)
                    eu = epool.tile([P, C], F32, tag="eu")
                    nc.scalar.activation(eu[:], o1[:], AF.Exp)
                    sse = epool.tile([P, 1], F32, tag="sse")
                    nc.vector.reduce_sum(out=sse[:], in_=eu[:],
                                         axis=mybir.AxisListType.X)
                    nc.scalar.activation(sse[:], sse[:], AF.Ln)
                    nc.vector.tensor_tensor(
                        out=st[:, g * C:(g + 1) * C], in0=o1[:],
                        in1=sse[:].to_broadcast([P, C]),
                        op=mybir.AluOpType.subtract)
                if g == STAGE_G - 1 or t == NTILE - 1:
                    flush_stage(t)

            for rep in range(reps):
              for sc in range(nsup):
                  gh = dpool.tile([P, KSUP * wh], BF16, tag="gh")
                  nc.sync.dma_start(out=gh[:],
                                    in_=gh_in[:, sc * KSUP * wh:(sc + 1) * KSUP * wh])
                  ssb = dpool.tile([P, KSUP * ws], BF16, tag="ssb")
                  nc.sync.dma_start(out=ssb[:],
                                    in_=ss_in[:, sc * KSUP * ws:(sc + 1) * KSUP * ws])
                  adb = dpool.tile([P, KSUP * ws], BF16, tag="adb")
                  nc.sync.dma_start(out=adb[:],
                                    in_=ad_in[:, sc * KSUP * ws:(sc + 1) * KSUP * ws])
                  wlb = dpool.tile([P, KSUP], BF16, tag="wlb")
                  nc.sync.dma_start(out=wlb[:],
                                    in_=wl_in[:, sc * KSUP:(sc + 1) * KSUP])

                  mt = wpool.tile([P, KSUP * WIN], BF16, tag="mt")
                  nc.vector.tensor_tensor(
                      out=mt[:].rearrange("p (k n) -> p k n", k=KSUP),
                      in0=wlb[:, :, None].to_broadcast([P, KSUP, WIN]),
                      in1=iota_f[:, None, :].to_broadcast([P, KSUP, WIN]),
                      op=mybir.AluOpType.is_equal)
                  e8 = wpool.tile([P, KSUP * ws], F32, tag="e8")
                  nc.vector.tensor_tensor(out=e8[:], in0=ssb[:], in1=adb[:],
                                          op=mybir.AluOpType.add)
                  nc.scalar.activation(e8[:], e8[:], AF.Lrelu, alpha=NEG)
                  wall = wpool.tile([P, KSUP * R], BF16, tag="wall")
                  wall_v = wall[:].rearrange("p (k r) -> p k r", k=KSUP)
                  nc.scalar.activation(
                      wall_v[:, :, wh:R],
                      e8[:].rearrange("p (k s) -> p k s", k=KSUP), AF.Exp)
                  nc.vector.tensor_tensor(
                      out=wall_v[:, :, 0:wh].rearrange(
                          "p k (h d) -> p k h d", h=(H if layer == 1 else 1)),
                      in0=gh[:].rearrange("p (k h d) -> p k h d",
                                          k=KSUP, h=(H if layer == 1 else 1)),
                      in1=wall_v[:, :, wh:R][:, :, :, None].to_broadcast(
                          [P, KSUP, (H if layer == 1 else 1),
                           (D if layer == 1 else C)]),
                      op=mybir.AluOpType.mult)

                  for j in range(KSUP):
                      if nomm:
                          break
                      cidx = sc * KSUP + j
                      t = int(chunk_tile[cidx])
                      lhs = mt[:, j * WIN:(j + 1) * WIN]
                      rhs = wall[:, j * R:(j + 1) * R]
                      if t < 0:
                          nc.tensor.matmul(trash[0:WIN, :], lhs, rhs,
                                           start=not trash_used[0], stop=False,
                                           skip_group_check=True)
                          trash_used[0] = True
                          continue
                      b = int(chunk_b[cidx])
                      if chunk_start[cidx]:
                          if b == 0:
                              psum_by_tile[t] = ppool.tile([P, R], F32, tag="acc", name=f"acc{t}")
                          ps = psum_by_tile[t]
                      else:
                          ps = psum_by_tile[t]
                      nc.tensor.matmul(
                          ps[b:b + WIN, :], lhs, rhs,
                          start=bool(chunk_start[cidx]),
                          stop=bool(chunk_stop[cidx]),
                          skip_group_check=True)
                      te = int(chunk_epi[cidx])
                      if te >= 0:
                          epilogue(te, psum_by_tile.pop(te))
    nc.compile()
    return nc


# ---------------------------------------------------------------- orchestration

def make_A_inputs(inputs):
    x = np.asarray(inputs["x"], np.float32)
    in_maps = []
    for k in range(NCORES):
        lo = k * NSHARD
        xs = np.ascontiguousarray(
            x[lo:lo + NSHARD].T.reshape(4, P, NSHARD)
            .transpose(1, 0, 2).reshape(P, 4 * NSHARD))
        in_maps.append({"XT": xs,
                        "W1": np.ascontiguousarray(np.asarray(inputs["W1"])),
                        "A1S": np.asarray(inputs["a1_src"]).reshape(-1),
                        "A1D": np.asarray(inputs["a1_dst"]).reshape(-1)})
    return in_maps


def make_B_inputs(g, ghs, sss, ads, inputs):
    return [{"GH": ghs[k], "SS": sss[k], "AD": ads[k], "WL": g.wloc[k],
             "B1": np.asarray(inputs["b1"]),
             "W2": np.ascontiguousarray(np.asarray(inputs["W2"])),
             "A2S": np.asarray(inputs["a2_src"]).reshape(-1),
             "A2D": np.asarray(inputs["a2_dst"]).reshape(-1)}
            for k in range(NCORES)]


def make_C_inputs(g, ghs, sss, ads, inputs):
    return [{"GH": ghs[k], "SS": sss[k], "AD": ads[k], "WL": g.wloc[k],
             "B2": np.asarray(inputs["b2"])} for k in range(NCORES)]


class GAT:
    def __init__(self, edge_index):
        self.meta, self.placements, self.wloc = build_structure(edge_index)
        self.ncA = build_A()
        self.ncB = build_edge_launch(self.meta, 1)
        self.ncC = build_edge_launch(self.meta, 2)

    def run(self, x, W1, a1_src, a1_dst, b1, W2, a2_src, a2_dst, b2,
            runner=run_bass_kernel_spmd):
        kt = self.meta["kt"]
        inputs = dict(x=x, W1=W1, a1_src=a1_src, a1_dst=a1_dst, b1=b1,
                      W2=W2, a2_src=a2_src, a2_dst=a2_dst, b2=b2)
        # ---- launch A
        in_maps = make_A_inputs(inputs)
        resA = runner(self.ncA, in_maps, core_ids=list(range(NCORES))).results
        th = np.concatenate([r["TH"][:NSHARD] for r in resA])
        tsrc = np.concatenate([r["TS"][:NSHARD] for r in resA])
        tad = np.concatenate([r["TAD"][:NSHARD] for r in resA])

        # ---- streams for B (host layout)
        ghs, sss, ads = make_streams(self.placements, kt, th, tsrc, tad, HD, H)
        in_maps = make_B_inputs(self, ghs, sss, ads, inputs)
        resB = runner(self.ncB, in_maps, core_ids=list(range(NCORES))).results
        t2 = np.concatenate([r["T2"][:NSHARD] for r in resB])
        t2h, t2s, t2ad = t2[:, 0:C], t2[:, C:C + 1], t2[:, C + 1:C + 2]

        # ---- streams for C
        ghs, sss, ads = make_streams(self.placements, kt, t2h, t2s, t2ad, C, 1)
        in_maps = make_C_inputs(self, ghs, sss, ads, inputs)
        resC = runner(self.ncC, in_maps, core_ids=list(range(NCORES))).results
        return np.concatenate([r["OUT"][:NSHARD] for r in resC])


def kernel(x, edge_index, W1, a1_src, a1_dst, b1, W2, a2_src, a2_dst, b2):
    g = GAT(np.asarray(edge_index))
    return g.run(np.asarray(x, np.float32), np.asarray(W1), np.asarray(a1_src),
                 np.asarray(a1_dst), np.asarray(b1), np.asarray(W2),
                 np.asarray(a2_src), np.asarray(a2_dst), np.asarray(b2))

